# revision 3
# baseline (speedup 1.0000x reference)
"""Bass/Tile TRN2 kernel for nn_MultiHeadSeqAttention_82789789597729.

Math: the reference's softmax / positional scores are dead code -- its output
is exactly  out = concat_h(q_h @ k_h^T @ v_h) @ Wo^T  with no nonlinearity.
By associativity  q (k^T v)  replaces the [M,M] score matrix with a [D,D]
one, collapsing ~69 GFLOP to ~26 GFLOP.

Sharding: tensor-parallel over heads (4 heads / core) x data-parallel over
batch (B=2) -> 8 cores. Each core computes a full-M partial output for its
head group; the host sums the 4 partials per batch (row-parallel unshard).
"""

import numpy as np
import ml_dtypes

import concourse.bass as bass
import concourse.mybir as mybir
import concourse.tile as tile
from concourse.bass_utils import run_bass_kernel_spmd
from concourse.vector_clock import ScopedClock
import bass_rust

B, M, H, K, D = 2, 2048, 1024, 16, 64
N_CORES = 8
HPC = 4           # heads per core
CC = HPC * D      # 256 local feature columns per core
P = 128

# matmul dtype mode: "f32" (safe), "f32r" (full-rate fp32 storage), "bf16"
MM_DT = "f32r"


# --- workaround: this walrus rejects multi-wait Drain instructions, so split
# --- the TileContext exit drain into one single-wait drain per proc.
def _split_drain_and_barrier(self, tick_clock, wait_clock):
    n_procs = len(list(tick_clock.global_clock))
    for p, t in enumerate(tick_clock.global_clock):
        if t <= 0:
            continue
        single = bass_rust.VectorClock(
            [t if i == p else 0 for i in range(n_procs)]
        )
        d = self.nc.sync.drain()
        wait_clock.add_sem_waits(d.ins, ScopedClock({None: single}))
    self.nc.all_engine_barrier()
    popped = self.nc._tile_sem_poison_stack.pop()
    assert popped is self._sem_poison
    self.nc.clear_and_free_semaphores(list(self.sems.allocated().values()))
    self.nc.all_engine_barrier()


# --- workaround: the same walrus caps sync waits at 1 per instruction
# --- (2 for EventSemaphore). Tile's wait-assignment can attach more; hoist
# --- the extras onto single-wait nop carriers emitted just before.
_ORIG_COMMIT_AND_LOWER = tile.TileContext._commit_and_lower


def _wait_split_commit_and_lower(self, inst, original_block, old_bb_map,
                                 bb_to_exit_bb):
    si = inst.sync_info
    cap = 2 if isinstance(inst, mybir.InstEventSemaphore) else 1
    ow = list(si.on_wait) if si is not None and si.on_wait else []
    if len(ow) > cap and inst.is_executable():
        for w in ow[:-cap]:
            carrier = self.nc.engines[inst.engine].nop(nofuse=True)
            carrier.ins.sync_info = bass_rust.SyncInfo(
                on_wait=[w], on_update=[]
            )
        inst.sync_info = bass_rust.SyncInfo(
            on_wait=ow[-cap:], on_update=list(si.on_update or [])
        )
    return _ORIG_COMMIT_AND_LOWER(
        self, inst, original_block, old_bb_map, bb_to_exit_bb
    )


if not getattr(tile.TileContext, "_split_drain_patched", False):
    tile.TileContext._drain_and_barrier = _split_drain_and_barrier
    tile.TileContext._commit_and_lower = _wait_split_commit_and_lower
    tile.TileContext._split_drain_patched = True


def _mm(nc, out, lhsT, rhs, start, stop):
    nc.tensor.matmul(out, lhsT, rhs, start=start, stop=stop)


def _build_nc():
    if MM_DT == "bf16":
        io_dt = mybir.dt.bfloat16
    elif MM_DT == "f32r":
        io_dt = mybir.dt.float32r
    else:
        io_dt = mybir.dt.float32
    f32 = mybir.dt.float32

    nc = bass.Bass()
    hT = nc.dram_tensor("hT", [H, M], io_dt, kind="ExternalInput")
    hcT = nc.dram_tensor("hcT", [H, M], io_dt, kind="ExternalInput")
    wqT = nc.dram_tensor("wqT", [H, CC], io_dt, kind="ExternalInput")
    wkT = nc.dram_tensor("wkT", [H, CC], io_dt, kind="ExternalInput")
    wvT = nc.dram_tensor("wvT", [H, CC], io_dt, kind="ExternalInput")
    woT = nc.dram_tensor("woT", [CC, H], io_dt, kind="ExternalInput")
    outp = nc.dram_tensor("out", [M, H], f32, kind="ExternalOutput")

    IT = H // P           # 8 contraction tiles over feature dim
    LT = M // P           # 16 tiles over sequence dim
    MC = M // 512         # 4 moving chunks over sequence dim
    DT = CC // P          # 2 partition tiles over local feature cols
    JC = H // 512         # 2 chunks over output feature dim

    x_bufs = 12 if MM_DT == "bf16" else 9

    with tile.TileContext(nc) as tc:
        with (
            tc.tile_pool(name="wp", bufs=1) as wp,
            tc.tile_pool(name="xp", bufs=x_bufs) as xp,
            tc.tile_pool(name="big", bufs=1) as big,
            tc.tile_pool(name="op", bufs=3) as op,
            tc.tile_pool(name="ps", bufs=6, space="PSUM") as ps,
        ):
            # weights
            wk_sb = wp.tile([P, IT, CC], io_dt, tag="wk")
            nc.sync.dma_start(
                out=wk_sb[:], in_=wkT.rearrange("(it p) c -> p it c", p=P)
            )
            wv_sb = wp.tile([P, IT, CC], io_dt, tag="wv")
            nc.sync.dma_start(
                out=wv_sb[:], in_=wvT.rearrange("(it p) c -> p it c", p=P)
            )
            wq_sb = wp.tile([P, IT, CC], io_dt, tag="wq")
            nc.sync.dma_start(
                out=wq_sb[:], in_=wqT.rearrange("(it p) c -> p it c", p=P)
            )
            wo_sb = wp.tile([D, HPC, H], io_dt, tag="wo")
            nc.sync.dma_start(
                out=wo_sb[:], in_=woT.rearrange("(hh p) j -> p hh j", p=D)
            )

            # h_cache^T stream tiles (slots later reused by h^T)
            hc_t = []
            for it in range(IT):
                t = xp.tile([P, M], io_dt, tag="xt")
                nc.sync.dma_start(out=t[:], in_=hcT[it * P:(it + 1) * P, :])
                hc_t.append(t)
            h_t = []
            for it in range(IT):
                t = xp.tile([P, M], io_dt, tag="xt")
                nc.sync.dma_start(out=t[:], in_=hT[it * P:(it + 1) * P, :])
                h_t.append(t)

            # persistent intermediates
            k_sb = big.tile([P, LT, CC], io_dt, tag="k")
            v_sb = big.tile([P, LT, CC], io_dt, tag="v")
            q_sb = big.tile([P, DT, M], io_dt, tag="q")
            at_sb = big.tile([D, HPC, D], io_dt, tag="at")
            c_sb = big.tile([P, DT, H], io_dt, tag="c")

            # --- stage P1: k, v projections (natural [l, d] layout)
            for lt in range(LT):
                for dst, w_sb in ((k_sb, wk_sb), (v_sb, wv_sb)):
                    acc = ps.tile([P, CC], f32, tag="ps")
                    for it in range(IT):
                        _mm(
                            nc, acc[:],
                            hc_t[it][:, lt * P:(lt + 1) * P],
                            w_sb[:, it, :],
                            start=(it == 0), stop=(it == IT - 1),
                        )
                    nc.vector.tensor_copy(dst[:, lt, :], acc[:])

            # --- stage P2: q^T projection ([du, m] layout)
            for dt_i in range(DT):
                for mc in range(MC):
                    acc = ps.tile([P, 512], f32, tag="ps")
                    for it in range(IT):
                        _mm(
                            nc, acc[:],
                            wq_sb[:, it, dt_i * P:(dt_i + 1) * P],
                            h_t[it][:, mc * 512:(mc + 1) * 512],
                            start=(it == 0), stop=(it == IT - 1),
                        )
                    nc.vector.tensor_copy(
                        q_sb[:, dt_i, mc * 512:(mc + 1) * 512], acc[:]
                    )

            # --- stage A: AT_h = v_h^T k_h  [dv, du], per head
            for hh in range(HPC):
                acc = ps.tile([D, D], f32, tag="ps")
                for lt in range(LT):
                    _mm(
                        nc, acc[:],
                        v_sb[:, lt, hh * D:(hh + 1) * D],
                        k_sb[:, lt, hh * D:(hh + 1) * D],
                        start=(lt == 0), stop=(lt == LT - 1),
                    )
                nc.vector.tensor_copy(at_sb[:, hh, :], acc[:])

            # --- stage C: C_h = A_h^T-applied rows of (A_h Wo_h^T) [du, j]
            for hh in range(HPC):
                prow = (hh % 2) * D
                for jc in range(JC):
                    acc = ps.tile([D, 512], f32, tag="ps")
                    _mm(
                        nc, acc[:],
                        at_sb[:, hh, :],
                        wo_sb[:, hh, jc * 512:(jc + 1) * 512],
                        start=True, stop=True,
                    )
                    nc.vector.tensor_copy(
                        c_sb[prow:prow + D, hh // 2, jc * 512:(jc + 1) * 512],
                        acc[:],
                    )

            # --- stage O: out[m, j] = sum_dt q^T[:, dt, m].T @ C[dt]
            for mt in range(LT):
                o_t = op.tile([P, H], f32, tag="o")
                for jc in range(JC):
                    acc = ps.tile([P, 512], f32, tag="ps")
                    for dt_i in range(DT):
                        _mm(
                            nc, acc[:],
                            q_sb[:, dt_i, mt * P:(mt + 1) * P],
                            c_sb[:, dt_i, jc * 512:(jc + 1) * 512],
                            start=(dt_i == 0), stop=(dt_i == DT - 1),
                        )
                    nc.vector.tensor_copy(
                        o_t[:, jc * 512:(jc + 1) * 512], acc[:]
                    )
                nc.sync.dma_start(
                    out=outp[mt * P:(mt + 1) * P, :], in_=o_t[:]
                )

    return nc


_NC_CACHE = {}


def _get_nc():
    if "nc" not in _NC_CACHE:
        _NC_CACHE["nc"] = _build_nc()
    return _NC_CACHE["nc"]


def _cast(a):
    a = np.ascontiguousarray(a)
    if MM_DT == "bf16":
        return a.astype(ml_dtypes.bfloat16)
    return a.astype(np.float32)


def make_in_maps(h, h_cache, Wq, Wk, Wv, Wo):
    in_maps = []
    for c in range(N_CORES):
        b, g = divmod(c, 4)
        cols = slice(g * CC, (g + 1) * CC)
        in_maps.append({
            "hT": _cast(h[b].T),
            "hcT": _cast(h_cache[b].T),
            "wqT": _cast(Wq[cols, :].T),
            "wkT": _cast(Wk[cols, :].T),
            "wvT": _cast(Wv[cols, :].T),
            "woT": _cast(Wo[:, cols].T),
        })
    return in_maps


def kernel(h, h_cache, key_pe, Wq, Wk, Wv, Wo, _bass_results=None):
    h = np.asarray(h)
    h_cache = np.asarray(h_cache)
    Wq, Wk, Wv, Wo = (np.asarray(a) for a in (Wq, Wk, Wv, Wo))
    nc = _get_nc()
    in_maps = make_in_maps(h, h_cache, Wq, Wk, Wv, Wo)
    res = run_bass_kernel_spmd(nc, in_maps, list(range(N_CORES)))
    if _bass_results is not None:
        _bass_results.append(res)
    out = np.zeros((B, M, H), np.float32)
    for c in range(N_CORES):
        out[c // 4] += res.results[c]["out"]
    return out


# revision 7
# speedup vs baseline: 1.3363x; 1.3363x over previous
"""Bass/Tile TRN2 kernel for nn_MultiHeadSeqAttention_82789789597729.

Math: the reference's softmax / positional scores are dead code -- its output
is exactly  out = concat_h(q_h @ k_h^T @ v_h) @ Wo^T  with no nonlinearity.
By associativity  q (k^T v)  replaces the [M,M] score matrix with a [D,D]
one, collapsing ~69 GFLOP to ~26 GFLOP.

Sharding: tensor-parallel over heads (4 heads / core) x data-parallel over
batch (B=2) -> 8 cores. Each core computes a full-M partial output for its
head group; the host sums the 4 partials per batch (row-parallel unshard).
"""

import numpy as np
import ml_dtypes

import concourse.bass as bass
import concourse.mybir as mybir
import concourse.tile as tile
from concourse.bass_utils import run_bass_kernel_spmd
from concourse.vector_clock import ScopedClock
import bass_rust

B, M, H, K, D = 2, 2048, 1024, 16, 64
N_CORES = 8
HPC = 4           # heads per core
CC = HPC * D      # 256 local feature columns per core
P = 128

# matmul dtype mode: "f32" (safe), "f32r" (full-rate fp32 storage), "bf16"
MM_DT = "f16"


# --- workaround: this walrus rejects multi-wait Drain instructions, so split
# --- the TileContext exit drain into one single-wait drain per proc.
def _split_drain_and_barrier(self, tick_clock, wait_clock):
    n_procs = len(list(tick_clock.global_clock))
    for p, t in enumerate(tick_clock.global_clock):
        if t <= 0:
            continue
        single = bass_rust.VectorClock(
            [t if i == p else 0 for i in range(n_procs)]
        )
        d = self.nc.sync.drain()
        wait_clock.add_sem_waits(d.ins, ScopedClock({None: single}))
    self.nc.all_engine_barrier()
    popped = self.nc._tile_sem_poison_stack.pop()
    assert popped is self._sem_poison
    self.nc.clear_and_free_semaphores(list(self.sems.allocated().values()))
    self.nc.all_engine_barrier()


# --- workaround: the same walrus caps sync waits at 1 per instruction
# --- (2 for EventSemaphore). Tile's wait-assignment can attach more; hoist
# --- the extras onto single-wait nop carriers emitted just before.
_ORIG_COMMIT_AND_LOWER = tile.TileContext._commit_and_lower


def _wait_split_commit_and_lower(self, inst, original_block, old_bb_map,
                                 bb_to_exit_bb):
    si = inst.sync_info
    cap = 2 if isinstance(inst, mybir.InstEventSemaphore) else 1
    ow = list(si.on_wait) if si is not None and si.on_wait else []
    if len(ow) > cap and inst.is_executable():
        for w in ow[:-cap]:
            carrier = self.nc.engines[inst.engine].nop(nofuse=True)
            carrier.ins.sync_info = bass_rust.SyncInfo(
                on_wait=[w], on_update=[]
            )
        inst.sync_info = bass_rust.SyncInfo(
            on_wait=ow[-cap:], on_update=list(si.on_update or [])
        )
    return _ORIG_COMMIT_AND_LOWER(
        self, inst, original_block, old_bb_map, bb_to_exit_bb
    )


if not getattr(tile.TileContext, "_split_drain_patched", False):
    tile.TileContext._drain_and_barrier = _split_drain_and_barrier
    tile.TileContext._commit_and_lower = _wait_split_commit_and_lower
    tile.TileContext._split_drain_patched = True


def _mm(nc, out, lhsT, rhs, start, stop):
    nc.tensor.matmul(out, lhsT, rhs, start=start, stop=stop)


def _build_nc():
    if MM_DT == "bf16":
        io_dt = mybir.dt.bfloat16
    elif MM_DT == "f16":
        io_dt = mybir.dt.float16
    elif MM_DT == "f32r":
        io_dt = mybir.dt.float32r
    else:
        io_dt = mybir.dt.float32
    f32 = mybir.dt.float32

    nc = bass.Bass()
    hT = nc.dram_tensor("hT", [H, M], io_dt, kind="ExternalInput")
    hcT = nc.dram_tensor("hcT", [H, M], io_dt, kind="ExternalInput")
    wqT = nc.dram_tensor("wqT", [H, CC], io_dt, kind="ExternalInput")
    wkT = nc.dram_tensor("wkT", [H, CC], io_dt, kind="ExternalInput")
    wvT = nc.dram_tensor("wvT", [H, CC], io_dt, kind="ExternalInput")
    woT = nc.dram_tensor("woT", [CC, H], io_dt, kind="ExternalInput")
    outp = nc.dram_tensor("out", [M, H], f32, kind="ExternalOutput")

    IT = H // P           # 8 contraction tiles over feature dim
    LT = M // P           # 16 tiles over sequence dim
    MC = M // 512         # 4 moving chunks over sequence dim
    DT = CC // P          # 2 partition tiles over local feature cols
    JC = H // 512         # 2 chunks over output feature dim

    x_bufs = 12 if MM_DT in ("bf16", "f16") else 9

    with tile.TileContext(nc) as tc:
        with (
            tc.tile_pool(name="wp", bufs=1) as wp,
            tc.tile_pool(name="xp", bufs=x_bufs) as xp,
            tc.tile_pool(name="big", bufs=1) as big,
            tc.tile_pool(name="op", bufs=4) as op,
            tc.tile_pool(name="ps", bufs=6, space="PSUM") as ps,
        ):
            QC = 4                 # column chunks per x-tile DMA
            QW = M // QC           # 512 columns per chunk
            # weight tiles (filled by per-it sliced DMAs below)
            wk_sb = wp.tile([P, IT, CC], io_dt, tag="wk")
            wv_sb = wp.tile([P, IT, CC], io_dt, tag="wv")
            wq_sb = wp.tile([P, IT, CC], io_dt, tag="wq")
            wo_sb = wp.tile([D, HPC, H], io_dt, tag="wo")
            hc_t = [
                xp.tile([P, M], io_dt, tag="xt", name=f"hc_t{it}")
                for it in range(IT)
            ]
            h_t = [
                xp.tile([P, M], io_dt, tag="xt", name=f"h_t{it}")
                for it in range(IT)
            ]

            def load_w(w_sb, wT, it):
                nc.sync.dma_start(
                    out=w_sb[:, it, :], in_=wT[it * P:(it + 1) * P, :]
                )

            def load_x(tiles, xT, it, qc):
                nc.sync.dma_start(
                    out=tiles[it][:, qc * QW:(qc + 1) * QW],
                    in_=xT[it * P:(it + 1) * P, qc * QW:(qc + 1) * QW],
                )

            # DMA issue order = criticality order, interleaved across queues
            for it in range(IT):
                load_w(wk_sb, wkT, it)
                load_x(hc_t, hcT, it, 0)
            for it in range(IT):
                load_w(wv_sb, wvT, it)
                load_x(hc_t, hcT, it, 1)
            for it in range(IT):
                load_x(hc_t, hcT, it, 2)
            for it in range(IT):
                load_x(hc_t, hcT, it, 3)
            for it in range(IT):
                load_w(wq_sb, wqT, it)
                for qc in range(QC):
                    load_x(h_t, hT, it, qc)
            nc.sync.dma_start(
                out=wo_sb[:], in_=woT.rearrange("(hh p) j -> p hh j", p=D)
            )

            # persistent intermediates
            k_sb = big.tile([P, LT, CC], io_dt, tag="k")
            v_sb = big.tile([P, LT, CC], io_dt, tag="v")
            q_sb = big.tile([P, DT, M], io_dt, tag="q")
            at_sb = big.tile([D, HPC, D], io_dt, tag="at")
            c_sb = big.tile([P, DT, H], io_dt, tag="c")

            # --- stage P1: k, v projections (natural [l, d] layout)
            for lt in range(LT):
                for dst, w_sb in ((k_sb, wk_sb), (v_sb, wv_sb)):
                    acc = ps.tile([P, CC], f32, tag="ps")
                    for it in range(IT):
                        _mm(
                            nc, acc[:],
                            hc_t[it][:, lt * P:(lt + 1) * P],
                            w_sb[:, it, :],
                            start=(it == 0), stop=(it == IT - 1),
                        )
                    nc.vector.tensor_copy(dst[:, lt, :], acc[:])

            # --- stage A: AT_h = v_h^T k_h  [dv, du], per head
            for hh in range(HPC):
                acc = ps.tile([D, D], f32, tag="ps")
                for lt in range(LT):
                    _mm(
                        nc, acc[:],
                        v_sb[:, lt, hh * D:(hh + 1) * D],
                        k_sb[:, lt, hh * D:(hh + 1) * D],
                        start=(lt == 0), stop=(lt == LT - 1),
                    )
                nc.vector.tensor_copy(at_sb[:, hh, :], acc[:])

            # --- stage C: rows of (A_h Wo_h^T) [du, j]
            for hh in range(HPC):
                prow = (hh % 2) * D
                for jc in range(JC):
                    acc = ps.tile([D, 512], f32, tag="ps")
                    _mm(
                        nc, acc[:],
                        at_sb[:, hh, :],
                        wo_sb[:, hh, jc * 512:(jc + 1) * 512],
                        start=True, stop=True,
                    )
                    nc.vector.tensor_copy(
                        c_sb[prow:prow + D, hh // 2, jc * 512:(jc + 1) * 512],
                        acc[:],
                    )

            # --- stage P2 + O interleaved per 512-column chunk of m:
            # q^T chunk [du, m-chunk], then out rows for those m
            for mc in range(MC):
                for dt_i in range(DT):
                    acc = ps.tile([P, 512], f32, tag="ps")
                    for it in range(IT):
                        _mm(
                            nc, acc[:],
                            wq_sb[:, it, dt_i * P:(dt_i + 1) * P],
                            h_t[it][:, mc * 512:(mc + 1) * 512],
                            start=(it == 0), stop=(it == IT - 1),
                        )
                    nc.vector.tensor_copy(
                        q_sb[:, dt_i, mc * 512:(mc + 1) * 512], acc[:]
                    )
                for mt in range(mc * 4, (mc + 1) * 4):
                    o_t = op.tile([P, H], f32, tag="o")
                    for jc in range(JC):
                        acc = ps.tile([P, 512], f32, tag="ps")
                        for dt_i in range(DT):
                            _mm(
                                nc, acc[:],
                                q_sb[:, dt_i, mt * P:(mt + 1) * P],
                                c_sb[:, dt_i, jc * 512:(jc + 1) * 512],
                                start=(dt_i == 0), stop=(dt_i == DT - 1),
                            )
                        nc.vector.tensor_copy(
                            o_t[:, jc * 512:(jc + 1) * 512], acc[:]
                        )
                        nc.sync.dma_start(
                            out=outp[mt * P:(mt + 1) * P,
                                     jc * 512:(jc + 1) * 512],
                            in_=o_t[:, jc * 512:(jc + 1) * 512],
                        )

    return nc


_NC_CACHE = {}


def _get_nc():
    if "nc" not in _NC_CACHE:
        _NC_CACHE["nc"] = _build_nc()
    return _NC_CACHE["nc"]


def _cast(a):
    a = np.ascontiguousarray(a)
    if MM_DT == "bf16":
        return a.astype(ml_dtypes.bfloat16)
    if MM_DT == "f16":
        return a.astype(np.float16)
    return a.astype(np.float32)


def make_in_maps(h, h_cache, Wq, Wk, Wv, Wo):
    in_maps = []
    for c in range(N_CORES):
        b, g = divmod(c, 4)
        cols = slice(g * CC, (g + 1) * CC)
        in_maps.append({
            "hT": _cast(h[b].T),
            "hcT": _cast(h_cache[b].T),
            "wqT": _cast(Wq[cols, :].T),
            "wkT": _cast(Wk[cols, :].T),
            "wvT": _cast(Wv[cols, :].T),
            "woT": _cast(Wo[:, cols].T),
        })
    return in_maps


def kernel(h, h_cache, key_pe, Wq, Wk, Wv, Wo, _bass_results=None):
    h = np.asarray(h)
    h_cache = np.asarray(h_cache)
    Wq, Wk, Wv, Wo = (np.asarray(a) for a in (Wq, Wk, Wv, Wo))
    nc = _get_nc()
    in_maps = make_in_maps(h, h_cache, Wq, Wk, Wv, Wo)
    res = run_bass_kernel_spmd(nc, in_maps, list(range(N_CORES)))
    if _bass_results is not None:
        _bass_results.append(res)
    out = np.zeros((B, M, H), np.float32)
    for c in range(N_CORES):
        out[c // 4] += res.results[c]["out"]
    return out
